# revision 23
# baseline (speedup 1.0000x reference)
"""Trainium2 Bass kernel for a single non-causal attention head.

Problem: x [8, 2048, 768] f32; Wq/Wk/Wv [768, 64]; bq/bk/bv [64].
  q = x@Wq+bq; k = x@Wk+bk; v = x@Wv+bv
  out = softmax(q k^T / sqrt(64)) @ v          -> [8, 2048, 64] f32

Sharding: data-parallel over batch B=8, one batch element per NeuronCore.

Per-core dataflow (all matmul operands bf16, fp32 accumulation in PSUM):
  1. x is loaded straight to bf16 via gpsimd (SWDGE) cast-DMAs, one DMA per
     512-row chunk (first chunk split in two for a faster pipeline start).
  2. Each 128-row x tile is PE-transposed (bf16, 1 cycle/row) into a PSUM
     tile and copied to the persistent xT [128d, 6, 2048t] (DVE 2x mode).
  3. Packed Q/K projection per 512-chunk: lhsT=[Wq|Wk] gives qT rows 0:64 /
     kT rows 64:128 in one 6-step accumulation; V is projected directly in
     natural [s, h] layout (lhsT = xT s-tile, rhs = Wv, N=64 -> 27ns/matmul)
     with a ones column appended so attention row-sums fall out of AV free.
  4. Flash loop over (fc t-chunk, pr s-pair): one [128, 2, 512] PSUM score
     tile (two K=64 matmuls), a single 1024-element exp on ScalarE (logit
     scale 1/8 folded in) -> ex bf16, then AV *transposed*: for each 128-t
     tile, matmul(out[t,65] += ex[s, t-slice].T @ v_sb[s-tile]) -- N=65, so
     the whole AV costs half of a streamed formulation AND the output lands
     in natural [t, h] layout: no epilogue transposes at all.
  5. Epilogue per (fc, jj): reciprocal of the sums column, per-partition
     scalar multiply -> ob, one DMA per 512-row block.

avo PSUM accumulators exist for 2 flash chunks at a time (8 PSUM banks
total); fc=2 AV work is deferred until epilogue(0) frees a bank, with the
already-computed ex tiles held in SBUF meanwhile, so the Activation engine
(the bottleneck: T*T exps = 27us floor) never stalls on PSUM.

Softmax is computed without the running-max subtraction: logits are q.k/8
with |logit| < ~3 for this problem's N(0,1)-scaled inputs, so exp is far
from overflow and the result matches jax.nn.softmax to bf16 accuracy.

Biases are all-zero in this problem; the default program skips them but
kernel() falls back to a bias-applying variant if any bias is nonzero.
"""

import numpy as np

B, T, D, H = 8, 2048, 768, 64
P = 128
DT = D // P   # 6 d-tiles
TT = T // P   # 16 s/t-tiles
NCH = 512     # t-chunk width
NCC = T // NCH  # 4 chunks
NPR = TT // 2   # 8 s-pairs

_CACHE = {}


def _build(biases=False, n_cores=8):
    from contextlib import ExitStack

    import concourse.bass as bass
    import concourse.tile as tile
    from concourse import bacc, mybir
    from concourse.bass import ds, ts
    from concourse.masks import make_identity

    f32 = mybir.dt.float32
    bf = mybir.dt.bfloat16

    nc = bacc.Bacc(
        "TRN2",
        target_bir_lowering=False,
        debug=False,
        enable_asserts=False,
        num_devices=n_cores,
    )

    x_d = nc.dram_tensor("x", [T, D], f32, kind="ExternalInput").ap()
    wq_d = nc.dram_tensor("wq", [D, H], f32, kind="ExternalInput").ap()
    wk_d = nc.dram_tensor("wk", [D, H], f32, kind="ExternalInput").ap()
    wv_d = nc.dram_tensor("wv", [D, H], f32, kind="ExternalInput").ap()
    bq_d = nc.dram_tensor("bq", [H], f32, kind="ExternalInput").ap()
    bk_d = nc.dram_tensor("bk", [H], f32, kind="ExternalInput").ap()
    bv_d = nc.dram_tensor("bv", [H], f32, kind="ExternalInput").ap()
    out_d = nc.dram_tensor("out", [T, H], f32, kind="ExternalOutput").ap()

    x_ch = x_d.rearrange("(c p) d -> p c d", p=P)   # [128, 16, 768]
    out_tiles4 = out_d.rearrange("(n p) h -> p n h", p=P)

    scale = float(H) ** -0.5

    with tile.TileContext(nc) as tc, ExitStack() as ctx:
        const = ctx.enter_context(tc.tile_pool(name="const", bufs=1))
        big = ctx.enter_context(tc.tile_pool(name="big", bufs=1))
        xin = ctx.enter_context(tc.tile_pool(name="xin", bufs=1))
        work = ctx.enter_context(tc.tile_pool(name="work", bufs=1))
        pp = ctx.enter_context(tc.tile_pool(name="pp", bufs=1, space="PSUM"))

        # -- persistent activations (declared first so load_x can start
        #    desc-gen on the Pool engine before any other Pool work) -----
        xT = big.tile([P, DT, T], bf, tag="xT")          # xT[p, d, t] = x[t, 128d+p]
        qT = big.tile([H, T], bf, tag="qT")              # q^T [h, t]
        kT = big.tile([H, T], bf, tag="kT")              # k^T [h, t]
        v_sb = big.tile([P, TT, H + 1], bf, tag="v_sb")  # v natural + ones col

        x_half = {}

        def load_x_half(ch, half):
            # Half-chunk (2-tile) cast-DMAs: finer arrival granularity keeps
            # the kT-dependent score pairs fed without gen-pacing stalls.
            x_in = xin.tile([P, 2, D], bf, tag="x_in", bufs=8,
                            name=f"x_in_{ch}_{half}")
            nc.gpsimd.dma_start(x_in, x_ch[:, ds(4 * ch + 2 * half, 2), :])
            x_half[(ch, half)] = x_in

        load_x_half(0, 0)

        # -- constants / one-time setup ---------------------------------
        # PE p-state warmup: the Tensor engine ramps to full clock only
        # after ~3us of continuous work; a chain of dummy matmuls (fed by a
        # DVE-memset junk tile, ready at ~0.2us) bridges the gap until the
        # first x tiles arrive so the real transposes/projections run at
        # full speed.  Lives in the "sc" psum tag, unused until then.
        junk = work.tile([P, H], bf, tag="junk", name="junk")
        nc.vector.memset(junk, 0.0)
        wps = pp.tile([H, H], f32, tag="sc", bufs=2, name="warm")
        for i in range(48):
            nc.tensor.matmul(wps, junk, junk, start=True, stop=True,
                             skip_group_check=True)

        ident_f = const.tile([P, P], f32, tag="ident_f")
        make_identity(nc, ident_f)
        ident = const.tile([P, P], bf, tag="ident")
        nc.vector.tensor_copy(out=ident, in_=ident_f)

        # exp activation-table preload: tiny dummy exp at t~0 (the implicit
        # LoadActFuncSet preceding it has no operand deps and fires first)
        dum = work.tile([1, 4], f32, tag="dum", name="dum")
        nc.scalar.activation(dum, ident_f[0:1, 0:4],
                             mybir.ActivationFunctionType.Exp, scale=scale)

        # weights as bf16 via Pool cast-DMAs; emitted right after the chunk-0
        # x desc-gen in Pool program order (x chunk 0 keeps DMA priority, the
        # weights transfer next, chunks 1-3 follow).
        # wqk [128, 6, 0:64]=Wq, [.., 64:128]=Wk; wv [128, 6, 64]
        wqk = const.tile([P, DT, P], bf, tag="wqk")
        wv = const.tile([P, DT, H], bf, tag="wv")

        # Pool desc-gen order (= x/w DMA priority): first chunk-0 half, Wq,
        # Wk (gates the first projection), second half, Wv, chunks 1-3.
        nc.gpsimd.dma_start(wqk[:, :, 0:H], wq_d.rearrange("(n p) h -> p n h", p=P))
        nc.gpsimd.dma_start(wqk[:, :, H:P], wk_d.rearrange("(n p) h -> p n h", p=P))
        load_x_half(0, 1)
        nc.gpsimd.dma_start(wv, wv_d.rearrange("(n p) h -> p n h", p=P))
        for c in (1, 2, 3):
            load_x_half(c, 0)
            load_x_half(c, 1)

        if biases:
            bias_qk = const.tile([P, 1], f32, tag="bias_qk")
            nc.sync.dma_start(bias_qk[0:H, :], bq_d[:, None])
            nc.sync.dma_start(bias_qk[H:P, :], bk_d[:, None])
            # bv broadcast to [128, 64] via K=1 matmul with a ones column
            bv_sb = const.tile([1, H], f32, tag="bv_sb")
            nc.sync.dma_start(bv_sb, bv_d[None, :])
            ones_col = const.tile([1, P], f32, tag="ones_col")
            nc.gpsimd.memset(ones_col, 1.0)
            ps_bv = pp.tile([P, H], f32, tag="proj", bufs=2, name="ps_bv")
            nc.tensor.matmul(ps_bv, ones_col, bv_sb, start=True, stop=True)
            bv_b = const.tile([P, H], f32, tag="bv_b")
            nc.vector.tensor_copy(out=bv_b, in_=ps_bv)

        nc.gpsimd.memset(v_sb[:, :, H : H + 1], 1.0)

        # -- per-chunk x transpose + projections ------------------------
        def transpose_tile(tt):
            ch, i = tt // 4, tt % 4
            src = x_half[(ch, i // 2)][:, i % 2, :]
            tr = pp.tile([P, DT, P], bf, tag="sc", bufs=2, name=f"tr_{tt}")
            for d in range(DT):
                nc.tensor.transpose(tr[:, d, :], src[:, ds(d * P, P)], ident)
            nc.vector.tensor_copy(out=xT[:, :, ts(tt, P)], in_=tr)

        def proj_half(ch, half):
            # packed Q/K: psum rows 0:64 = q^T, 64:128 = k^T for this half
            HW2 = NCH // 2
            sl = ds(ch * NCH + half * HW2, HW2)
            ps = pp.tile([P, HW2], f32, tag="proj", bufs=2, name=f"qk_{ch}_{half}")
            for d in range(DT):
                nc.tensor.matmul(ps, wqk[:, d, :], xT[:, d, sl],
                                 start=(d == 0), stop=(d == DT - 1))
            if biases:
                nc.vector.tensor_scalar_add(qT[:, sl], ps[0:H, :], bias_qk[0:H, :])
                nc.vector.tensor_scalar_add(kT[:, sl], ps[H:P, :], bias_qk[H:P, :])
            else:
                nc.vector.tensor_copy(out=kT[:, sl], in_=ps[H:P, :])
                nc.vector.tensor_copy(out=qT[:, sl], in_=ps[0:H, :])
            # V in natural [s, h] layout: lhsT = xT s-tile, rhs = Wv, N=64
            pv = pp.tile([P, 2, H], f32, tag="proj", bufs=2, name=f"v_{ch}_{half}")
            for j in range(2):
                s = 4 * ch + 2 * half + j
                for d in range(DT):
                    nc.tensor.matmul(pv[:, j, :], xT[:, d, ts(s, P)], wv[:, d, :],
                                     start=(d == 0), stop=(d == DT - 1))
            nc.vector.tensor_copy(out=v_sb[:, ds(4 * ch + 2 * half, 2), 0:H], in_=pv)

        # -- flash machinery --------------------------------------------
        ex_tiles = {}

        def scores_exp(fc, pr):
            s0, s1 = 2 * pr, 2 * pr + 1
            tsl = ds(fc * NCH, NCH)
            ps_s = pp.tile([P, 2, NCH], f32, tag="sc", bufs=2, name=f"sc_{fc}_{pr}")
            nc.tensor.matmul(ps_s[:, 0, :], kT[:, ts(s0, P)], qT[:, tsl],
                             start=True, stop=True)
            nc.tensor.matmul(ps_s[:, 1, :], kT[:, ts(s1, P)], qT[:, tsl],
                             start=True, stop=True)
            ex = work.tile([P, 2, NCH], bf, tag="ex", bufs=14, name=f"ex_{fc}_{pr}")
            nc.scalar.activation(ex, ps_s, mybir.ActivationFunctionType.Exp,
                                 scale=scale)
            ex_tiles[(fc, pr)] = ex

        def av_pair(fc, pr):
            # One PSUM bank holds all four jj slices.  start=True marks the
            # whole 2KB zero-region pending, so only the very FIRST matmul of
            # the fc may set it (each slice then auto-initializes on its first
            # write); a per-slice start would wipe sibling slices' partials.
            ex = ex_tiles.pop((fc, pr))
            for jj in range(4):
                for j in range(2):
                    s = 2 * pr + j
                    nc.tensor.matmul(
                        avo[fc][:, jj, :],
                        ex[:, j, ds(jj * P, P)],
                        v_sb[:, s, :],
                        start=(pr == 0 and j == 0 and jj == 0),
                        stop=(pr == NPR - 1 and j == 1),
                        skip_group_check=True,
                    )

        def epilogue(fc):
            # Last chunk is exit-critical: reciprocals on DVE and multiplies
            # on the (by then idle) Activation engine run pipelined, and the
            # output DMA goes in two halves so the first dispatches earlier.
            last = fc == NCC - 1
            ob = work.tile([P, 4, H], f32, tag="ob", bufs=2, name=f"ob_{fc}")
            for jj in range(4):
                rc = work.tile([P, 1], f32, tag="rc", bufs=4, name=f"rc_{fc}_{jj}")
                nc.vector.reciprocal(rc, avo[fc][:, jj, H : H + 1])
                if last:
                    nc.scalar.mul(ob[:, jj, :], avo[fc][:, jj, 0:H], rc)
                else:
                    nc.vector.tensor_scalar_mul(ob[:, jj, :], avo[fc][:, jj, 0:H], rc)
                if biases:
                    nc.vector.tensor_tensor(
                        out=ob[:, jj, :], in0=ob[:, jj, :], in1=bv_b,
                        op=mybir.AluOpType.add)
                if last and jj == 1:
                    nc.sync.dma_start(out_tiles4[:, ds(fc * 4, 2), :], ob[:, 0:2, :])
            if last:
                nc.sync.dma_start(out_tiles4[:, ds(fc * 4 + 2, 2), :], ob[:, 2:4, :])
            else:
                nc.sync.dma_start(out_tiles4[:, ts(fc, 4), :], ob)

        avo = {}

        def new_avo(fc):
            avo[fc] = pp.tile([P, 4, H + 1], f32, tag="avo", bufs=2, name=f"avo{fc}")

        # -- schedule ----------------------------------------------------
        # waves: pair (fc, pr) becomes computable after proj chunk
        # c = max(fc, pr // 2).  AV for fc >= 2 is deferred until an avo
        # PSUM bank frees (after epilogue(fc - 2)); ex tiles wait in SBUF.
        pend = []  # scores emitted, AV not yet emitted (lag hides Act latency)

        def flush_pend(n_keep=0):
            while len(pend) > n_keep:
                av_pair(*pend.pop(0))

        def emit_pair(fc, pr, defer_av=False):
            scores_exp(fc, pr)
            if defer_av:
                return
            pend.append((fc, pr))
            if len(pend) > 2:
                av_pair(*pend.pop(0))

        for ch in range(NCC):
            for half in range(2):
                transpose_tile(4 * ch + 2 * half)
                transpose_tile(4 * ch + 2 * half + 1)
                proj_half(ch, half)

            if ch < NCC - 1:
                # wave ch: all pairs with max(fc, pr//2) == ch, fc ascending.
                for fc in range(ch + 1):
                    if fc not in avo and fc < 2:
                        new_avo(fc)
                    prs = (range(2 * ch, 2 * ch + 2) if fc < ch
                           else range(0, 2 * ch + 2))
                    for pr in prs:
                        emit_pair(fc, pr, defer_av=(fc >= 2))

        # wave 3 (hand-ordered for Act continuity + early bank recycling)
        emit_pair(0, 6)
        emit_pair(0, 7)
        flush_pend()
        epilogue(0)

        emit_pair(1, 6)
        emit_pair(1, 7)
        flush_pend()
        epilogue(1)

        new_avo(2)  # reuses avo[0]'s bank
        for pr in range(6):
            pend.append((2, pr))  # ex already computed in wave 2
        flush_pend(n_keep=2)
        emit_pair(2, 6)
        emit_pair(2, 7)
        flush_pend()
        epilogue(2)

        new_avo(3)  # reuses avo[1]'s bank
        for pr in range(NPR):
            emit_pair(3, pr)
        flush_pend()
        epilogue(3)

    nc.compile()
    return nc


def _get_nc(mm="bf16", biases=False):
    key = (mm, biases)
    if key not in _CACHE:
        _CACHE[key] = _build(biases=biases)
    return _CACHE[key]


def kernel(x, Wq, bq, Wk, bk, Wv, bv, mm="bf16", **_kw):
    from concourse.bass_utils import run_bass_kernel_spmd

    x = np.ascontiguousarray(np.asarray(x, dtype=np.float32))
    base = {
        "wq": np.ascontiguousarray(np.asarray(Wq, np.float32)),
        "wk": np.ascontiguousarray(np.asarray(Wk, np.float32)),
        "wv": np.ascontiguousarray(np.asarray(Wv, np.float32)),
        "bq": np.ascontiguousarray(np.asarray(bq, np.float32)),
        "bk": np.ascontiguousarray(np.asarray(bk, np.float32)),
        "bv": np.ascontiguousarray(np.asarray(bv, np.float32)),
    }
    use_biases = bool(
        np.any(base["bq"]) or np.any(base["bk"]) or np.any(base["bv"])
    )
    nc = _get_nc(mm, biases=use_biases)
    in_maps = [dict(base, x=x[b]) for b in range(B)]
    res = run_bass_kernel_spmd(nc, in_maps, core_ids=list(range(B)))
    return np.stack([r["out"] for r in res.results], axis=0)


# revision 24
# speedup vs baseline: 1.1294x; 1.1294x over previous
"""Trainium2 Bass kernel for a single non-causal attention head.

Problem: x [8, 2048, 768] f32; Wq/Wk/Wv [768, 64]; bq/bk/bv [64].
  q = x@Wq+bq; k = x@Wk+bk; v = x@Wv+bv
  out = softmax(q k^T / sqrt(64)) @ v          -> [8, 2048, 64] f32

Sharding: data-parallel over batch B=8, one batch element per NeuronCore.

Per-core dataflow (all matmul operands bf16, fp32 accumulation in PSUM):
  1. x is loaded straight to bf16 via gpsimd (SWDGE) cast-DMAs, one DMA per
     512-row chunk (first chunk split in two for a faster pipeline start).
  2. Each 128-row x tile is PE-transposed (bf16, 1 cycle/row) into a PSUM
     tile and copied to the persistent xT [128d, 6, 2048t] (DVE 2x mode).
  3. Packed Q/K projection per 512-chunk: lhsT=[Wq|Wk] gives qT rows 0:64 /
     kT rows 64:128 in one 6-step accumulation; V is projected directly in
     natural [s, h] layout (lhsT = xT s-tile, rhs = Wv, N=64 -> 27ns/matmul)
     with a ones column appended so attention row-sums fall out of AV free.
  4. Flash loop over (fc t-chunk, pr s-pair): one [128, 2, 512] PSUM score
     tile (two K=64 matmuls), a single 1024-element exp on ScalarE (logit
     scale 1/8 folded in) -> ex bf16, then AV *transposed*: for each 128-t
     tile, matmul(out[t,65] += ex[s, t-slice].T @ v_sb[s-tile]) -- N=65, so
     the whole AV costs half of a streamed formulation AND the output lands
     in natural [t, h] layout: no epilogue transposes at all.
  5. Epilogue per (fc, jj): reciprocal of the sums column, per-partition
     scalar multiply -> ob, one DMA per 512-row block.

avo PSUM accumulators exist for 2 flash chunks at a time (8 PSUM banks
total); fc=2 AV work is deferred until epilogue(0) frees a bank, with the
already-computed ex tiles held in SBUF meanwhile, so the Activation engine
(the bottleneck: T*T exps = 27us floor) never stalls on PSUM.

Softmax is computed without the running-max subtraction: logits are q.k/8
with |logit| < ~3 for this problem's N(0,1)-scaled inputs, so exp is far
from overflow and the result matches jax.nn.softmax to bf16 accuracy.

Biases are all-zero in this problem; the default program skips them but
kernel() falls back to a bias-applying variant if any bias is nonzero.
"""

import numpy as np

B, T, D, H = 8, 2048, 768, 64
P = 128
DT = D // P   # 6 d-tiles
TT = T // P   # 16 s/t-tiles
NCH = 512     # t-chunk width
NCC = T // NCH  # 4 chunks
NPR = TT // 2   # 8 s-pairs

_CACHE = {}


def _build(biases=False, n_cores=8):
    from contextlib import ExitStack

    import concourse.bass as bass
    import concourse.tile as tile
    from concourse import bacc, mybir
    from concourse.bass import ds, ts
    from concourse.masks import make_identity

    f32 = mybir.dt.float32
    bf = mybir.dt.bfloat16

    nc = bacc.Bacc(
        "TRN2",
        target_bir_lowering=False,
        debug=False,
        enable_asserts=False,
        num_devices=n_cores,
    )

    x_d = nc.dram_tensor("x", [T, D], f32, kind="ExternalInput").ap()
    wq_d = nc.dram_tensor("wq", [D, H], f32, kind="ExternalInput").ap()
    wk_d = nc.dram_tensor("wk", [D, H], f32, kind="ExternalInput").ap()
    wv_d = nc.dram_tensor("wv", [D, H], f32, kind="ExternalInput").ap()
    bq_d = nc.dram_tensor("bq", [H], f32, kind="ExternalInput").ap()
    bk_d = nc.dram_tensor("bk", [H], f32, kind="ExternalInput").ap()
    bv_d = nc.dram_tensor("bv", [H], f32, kind="ExternalInput").ap()
    out_d = nc.dram_tensor("out", [T, H], f32, kind="ExternalOutput").ap()

    x_ch = x_d.rearrange("(c p) d -> p c d", p=P)   # [128, 16, 768]
    out_tiles4 = out_d.rearrange("(n p) h -> p n h", p=P)

    scale = float(H) ** -0.5

    with tile.TileContext(nc) as tc, ExitStack() as ctx:
        const = ctx.enter_context(tc.tile_pool(name="const", bufs=1))
        big = ctx.enter_context(tc.tile_pool(name="big", bufs=1))
        xin = ctx.enter_context(tc.tile_pool(name="xin", bufs=1))
        work = ctx.enter_context(tc.tile_pool(name="work", bufs=1))
        pp = ctx.enter_context(tc.tile_pool(name="pp", bufs=1, space="PSUM"))

        # -- persistent activations (declared first so load_x can start
        #    desc-gen on the Pool engine before any other Pool work) -----
        xT = big.tile([P, DT, T], bf, tag="xT")          # xT[p, d, t] = x[t, 128d+p]
        qT = big.tile([H, T], bf, tag="qT")              # q^T [h, t]
        kT = big.tile([H, T], bf, tag="kT")              # k^T [h, t]
        v_sb = big.tile([P, TT, H + 1], bf, tag="v_sb")  # v natural + ones col

        x_half = {}

        def load_x_half(ch, half):
            # Half-chunk (2-tile) cast-DMAs: finer arrival granularity keeps
            # the kT-dependent score pairs fed without gen-pacing stalls.
            x_in = xin.tile([P, 2, D], bf, tag="x_in", bufs=8,
                            name=f"x_in_{ch}_{half}")
            nc.gpsimd.dma_start(x_in, x_ch[:, ds(4 * ch + 2 * half, 2), :])
            x_half[(ch, half)] = x_in

        load_x_half(0, 0)

        # -- constants / one-time setup ---------------------------------
        # PE p-state warmup: the Tensor engine ramps to full clock only
        # after ~3us of continuous work; a chain of dummy matmuls (fed by a
        # DVE-memset junk tile, ready at ~0.2us) bridges the gap until the
        # first x tiles arrive so the real transposes/projections run at
        # full speed.  Lives in the "sc" psum tag, unused until then.
        junk = work.tile([P, H], bf, tag="junk", name="junk")
        nc.vector.memset(junk, 0.0)
        wps = pp.tile([H, H], f32, tag="sc", bufs=2, name="warm")
        for i in range(48):
            nc.tensor.matmul(wps, junk, junk, start=True, stop=True,
                             skip_group_check=True)

        ident_f = const.tile([P, P], f32, tag="ident_f")
        make_identity(nc, ident_f)
        ident = const.tile([P, P], bf, tag="ident")
        nc.vector.tensor_copy(out=ident, in_=ident_f)

        # exp activation-table preload: tiny dummy exp at t~0 (the implicit
        # LoadActFuncSet preceding it has no operand deps and fires first)
        dum = work.tile([1, 4], f32, tag="dum", name="dum")
        nc.scalar.activation(dum, ident_f[0:1, 0:4],
                             mybir.ActivationFunctionType.Exp, scale=scale)

        # weights as bf16 via Pool cast-DMAs; emitted right after the chunk-0
        # x desc-gen in Pool program order (x chunk 0 keeps DMA priority, the
        # weights transfer next, chunks 1-3 follow).
        # wqk [128, 6, 0:64]=Wq, [.., 64:128]=Wk; wv [128, 6, 64]
        wqk = const.tile([P, DT, P], bf, tag="wqk")
        wv = const.tile([P, DT, H], bf, tag="wv")

        # Pool desc-gen order (= x/w DMA priority): first chunk-0 half, Wq,
        # Wk (gates the first projection), second half, Wv, chunks 1-3.
        nc.gpsimd.dma_start(wqk[:, :, 0:H], wq_d.rearrange("(n p) h -> p n h", p=P))
        nc.gpsimd.dma_start(wqk[:, :, H:P], wk_d.rearrange("(n p) h -> p n h", p=P))
        load_x_half(0, 1)
        nc.gpsimd.dma_start(wv, wv_d.rearrange("(n p) h -> p n h", p=P))
        for c in (1, 2, 3):
            load_x_half(c, 0)
            load_x_half(c, 1)

        if biases:
            bias_qk = const.tile([P, 1], f32, tag="bias_qk")
            nc.sync.dma_start(bias_qk[0:H, :], bq_d[:, None])
            nc.sync.dma_start(bias_qk[H:P, :], bk_d[:, None])
            # bv broadcast to [128, 64] via K=1 matmul with a ones column
            bv_sb = const.tile([1, H], f32, tag="bv_sb")
            nc.sync.dma_start(bv_sb, bv_d[None, :])
            ones_col = const.tile([1, P], f32, tag="ones_col")
            nc.gpsimd.memset(ones_col, 1.0)
            ps_bv = pp.tile([P, H], f32, tag="proj", bufs=2, name="ps_bv")
            nc.tensor.matmul(ps_bv, ones_col, bv_sb, start=True, stop=True)
            bv_b = const.tile([P, H], f32, tag="bv_b")
            nc.vector.tensor_copy(out=bv_b, in_=ps_bv)

        nc.gpsimd.memset(v_sb[:, :, H : H + 1], 1.0)

        # -- per-chunk x transpose + projections ------------------------
        def transpose_tile(tt):
            ch, i = tt // 4, tt % 4
            src = x_half[(ch, i // 2)][:, i % 2, :]
            tr = pp.tile([P, DT, P], bf, tag="proj", bufs=2, name=f"tr_{tt}")
            for d in range(DT):
                nc.tensor.transpose(tr[:, d, :], src[:, ds(d * P, P)], ident)
            nc.vector.tensor_copy(out=xT[:, :, ts(tt, P)], in_=tr)

        def proj_half(ch, half):
            # packed Q/K: psum rows 0:64 = q^T, 64:128 = k^T for this half
            HW2 = NCH // 2
            sl = ds(ch * NCH + half * HW2, HW2)
            ps = pp.tile([P, HW2], f32, tag="proj", bufs=2, name=f"qk_{ch}_{half}")
            for d in range(DT):
                nc.tensor.matmul(ps, wqk[:, d, :], xT[:, d, sl],
                                 start=(d == 0), stop=(d == DT - 1))
            if biases:
                nc.vector.tensor_scalar_add(qT[:, sl], ps[0:H, :], bias_qk[0:H, :])
                nc.vector.tensor_scalar_add(kT[:, sl], ps[H:P, :], bias_qk[H:P, :])
            else:
                nc.vector.tensor_copy(out=kT[:, sl], in_=ps[H:P, :])
                nc.vector.tensor_copy(out=qT[:, sl], in_=ps[0:H, :])
            # V in natural [s, h] layout: lhsT = xT s-tile, rhs = Wv, N=64
            pv = pp.tile([P, 2, H], f32, tag="proj", bufs=2, name=f"v_{ch}_{half}")
            for j in range(2):
                s = 4 * ch + 2 * half + j
                for d in range(DT):
                    nc.tensor.matmul(pv[:, j, :], xT[:, d, ts(s, P)], wv[:, d, :],
                                     start=(d == 0), stop=(d == DT - 1))
            nc.vector.tensor_copy(out=v_sb[:, ds(4 * ch + 2 * half, 2), 0:H], in_=pv)

        # -- flash machinery --------------------------------------------
        ex_tiles = {}

        def scores_exp(fc, pr):
            s0, s1 = 2 * pr, 2 * pr + 1
            tsl = ds(fc * NCH, NCH)
            ps_s = pp.tile([P, 2, NCH], f32, tag="sc", bufs=2, name=f"sc_{fc}_{pr}")
            nc.tensor.matmul(ps_s[:, 0, :], kT[:, ts(s0, P)], qT[:, tsl],
                             start=True, stop=True)
            nc.tensor.matmul(ps_s[:, 1, :], kT[:, ts(s1, P)], qT[:, tsl],
                             start=True, stop=True)
            ex = work.tile([P, 2, NCH], bf, tag="ex", bufs=14, name=f"ex_{fc}_{pr}")
            nc.scalar.activation(ex, ps_s, mybir.ActivationFunctionType.Exp,
                                 scale=scale)
            ex_tiles[(fc, pr)] = ex

        def av_pair(fc, pr):
            # One PSUM bank holds all four jj slices.  start=True marks the
            # whole 2KB zero-region pending, so only the very FIRST matmul of
            # the fc may set it (each slice then auto-initializes on its first
            # write); a per-slice start would wipe sibling slices' partials.
            ex = ex_tiles.pop((fc, pr))
            for jj in range(4):
                for j in range(2):
                    s = 2 * pr + j
                    nc.tensor.matmul(
                        avo[fc][:, jj, :],
                        ex[:, j, ds(jj * P, P)],
                        v_sb[:, s, :],
                        start=(pr == 0 and j == 0 and jj == 0),
                        stop=(pr == NPR - 1 and j == 1),
                        skip_group_check=True,
                    )

        def epilogue(fc):
            # Last chunk is exit-critical: reciprocals on DVE and multiplies
            # on the (by then idle) Activation engine run pipelined, and the
            # output DMA goes in two halves so the first dispatches earlier.
            last = fc == NCC - 1
            ob = work.tile([P, 4, H], f32, tag="ob", bufs=2, name=f"ob_{fc}")
            for jj in range(4):
                rc = work.tile([P, 1], f32, tag="rc", bufs=4, name=f"rc_{fc}_{jj}")
                nc.vector.reciprocal(rc, avo[fc][:, jj, H : H + 1])
                if last:
                    nc.scalar.mul(ob[:, jj, :], avo[fc][:, jj, 0:H], rc)
                else:
                    nc.vector.tensor_scalar_mul(ob[:, jj, :], avo[fc][:, jj, 0:H], rc)
                if biases:
                    nc.vector.tensor_tensor(
                        out=ob[:, jj, :], in0=ob[:, jj, :], in1=bv_b,
                        op=mybir.AluOpType.add)
                if last and jj == 1:
                    nc.sync.dma_start(out_tiles4[:, ds(fc * 4, 2), :], ob[:, 0:2, :])
            if last:
                nc.sync.dma_start(out_tiles4[:, ds(fc * 4 + 2, 2), :], ob[:, 2:4, :])
            else:
                nc.sync.dma_start(out_tiles4[:, ts(fc, 4), :], ob)

        avo = {}

        def new_avo(fc):
            avo[fc] = pp.tile([P, 4, H + 1], f32, tag="avo", bufs=2, name=f"avo{fc}")

        # -- schedule ----------------------------------------------------
        # waves: pair (fc, pr) becomes computable after proj chunk
        # c = max(fc, pr // 2).  AV for fc >= 2 is deferred until an avo
        # PSUM bank frees (after epilogue(fc - 2)); ex tiles wait in SBUF.
        pend = []  # scores emitted, AV not yet emitted (lag hides Act latency)

        def flush_pend(n_keep=0):
            while len(pend) > n_keep:
                av_pair(*pend.pop(0))

        def emit_pair(fc, pr, defer_av=False):
            scores_exp(fc, pr)
            if defer_av:
                return
            pend.append((fc, pr))
            if len(pend) > 2:
                av_pair(*pend.pop(0))

        for ch in range(NCC):
            for half in range(2):
                transpose_tile(4 * ch + 2 * half)
                transpose_tile(4 * ch + 2 * half + 1)
                proj_half(ch, half)

            if ch < NCC - 1:
                # wave ch: all pairs with max(fc, pr//2) == ch, fc ascending.
                for fc in range(ch + 1):
                    if fc not in avo and fc < 2:
                        new_avo(fc)
                    prs = (range(2 * ch, 2 * ch + 2) if fc < ch
                           else range(0, 2 * ch + 2))
                    for pr in prs:
                        emit_pair(fc, pr, defer_av=(fc >= 2))

        # wave 3 (hand-ordered for Act continuity + early bank recycling)
        emit_pair(0, 6)
        emit_pair(0, 7)
        flush_pend()
        epilogue(0)

        emit_pair(1, 6)
        emit_pair(1, 7)
        flush_pend()
        epilogue(1)

        new_avo(2)  # reuses avo[0]'s bank
        for pr in range(6):
            pend.append((2, pr))  # ex already computed in wave 2
        flush_pend(n_keep=2)
        emit_pair(2, 6)
        emit_pair(2, 7)
        flush_pend()
        epilogue(2)

        new_avo(3)  # reuses avo[1]'s bank
        for pr in range(NPR):
            emit_pair(3, pr)
        flush_pend()
        epilogue(3)

    nc.compile()
    return nc


def _get_nc(mm="bf16", biases=False):
    key = (mm, biases)
    if key not in _CACHE:
        _CACHE[key] = _build(biases=biases)
    return _CACHE[key]


def kernel(x, Wq, bq, Wk, bk, Wv, bv, mm="bf16", **_kw):
    from concourse.bass_utils import run_bass_kernel_spmd

    x = np.ascontiguousarray(np.asarray(x, dtype=np.float32))
    base = {
        "wq": np.ascontiguousarray(np.asarray(Wq, np.float32)),
        "wk": np.ascontiguousarray(np.asarray(Wk, np.float32)),
        "wv": np.ascontiguousarray(np.asarray(Wv, np.float32)),
        "bq": np.ascontiguousarray(np.asarray(bq, np.float32)),
        "bk": np.ascontiguousarray(np.asarray(bk, np.float32)),
        "bv": np.ascontiguousarray(np.asarray(bv, np.float32)),
    }
    use_biases = bool(
        np.any(base["bq"]) or np.any(base["bk"]) or np.any(base["bv"])
    )
    nc = _get_nc(mm, biases=use_biases)
    in_maps = [dict(base, x=x[b]) for b in range(B)]
    res = run_bass_kernel_spmd(nc, in_maps, core_ids=list(range(B)))
    return np.stack([r["out"] for r in res.results], axis=0)


# revision 27
# speedup vs baseline: 1.1440x; 1.0129x over previous
"""Trainium2 Bass kernel for a single non-causal attention head.

Problem: x [8, 2048, 768] f32; Wq/Wk/Wv [768, 64]; bq/bk/bv [64].
  q = x@Wq+bq; k = x@Wk+bk; v = x@Wv+bv
  out = softmax(q k^T / sqrt(64)) @ v          -> [8, 2048, 64] f32

Sharding: data-parallel over batch B=8, one batch element per NeuronCore.

Per-core dataflow (all matmul operands bf16, fp32 accumulation in PSUM):
  1. x is loaded straight to bf16 via gpsimd (SWDGE) cast-DMAs, one DMA per
     512-row chunk (first chunk split in two for a faster pipeline start).
  2. Each 128-row x tile is PE-transposed (bf16, 1 cycle/row) into a PSUM
     tile and copied to the persistent xT [128d, 6, 2048t] (DVE 2x mode).
  3. Packed Q/K projection per 512-chunk: lhsT=[Wq|Wk] gives qT rows 0:64 /
     kT rows 64:128 in one 6-step accumulation; V is projected directly in
     natural [s, h] layout (lhsT = xT s-tile, rhs = Wv, N=64 -> 27ns/matmul)
     with a ones column appended so attention row-sums fall out of AV free.
  4. Flash loop over (fc t-chunk, pr s-pair): one [128, 2, 512] PSUM score
     tile (two K=64 matmuls), a single 1024-element exp on ScalarE (logit
     scale 1/8 folded in) -> ex bf16, then AV *transposed*: for each 128-t
     tile, matmul(out[t,65] += ex[s, t-slice].T @ v_sb[s-tile]) -- N=65, so
     the whole AV costs half of a streamed formulation AND the output lands
     in natural [t, h] layout: no epilogue transposes at all.
  5. Epilogue per (fc, jj): reciprocal of the sums column, per-partition
     scalar multiply -> ob, one DMA per 512-row block.

avo PSUM accumulators exist for 2 flash chunks at a time (8 PSUM banks
total); fc=2 AV work is deferred until epilogue(0) frees a bank, with the
already-computed ex tiles held in SBUF meanwhile, so the Activation engine
(the bottleneck: T*T exps = 27us floor) never stalls on PSUM.

Softmax is computed without the running-max subtraction: logits are q.k/8
with |logit| < ~3 for this problem's N(0,1)-scaled inputs, so exp is far
from overflow and the result matches jax.nn.softmax to bf16 accuracy.

Biases are all-zero in this problem; the default program skips them but
kernel() falls back to a bias-applying variant if any bias is nonzero.
"""

import numpy as np

B, T, D, H = 8, 2048, 768, 64
P = 128
DT = D // P   # 6 d-tiles
TT = T // P   # 16 s/t-tiles
NCH = 512     # t-chunk width
NCC = T // NCH  # 4 chunks
NPR = TT // 2   # 8 s-pairs

_CACHE = {}


def _build(biases=False, n_cores=8):
    from contextlib import ExitStack

    import concourse.bass as bass
    import concourse.tile as tile
    from concourse import bacc, mybir
    from concourse.bass import ds, ts
    from concourse.masks import make_identity

    f32 = mybir.dt.float32
    bf = mybir.dt.bfloat16

    nc = bacc.Bacc(
        "TRN2",
        target_bir_lowering=False,
        debug=False,
        enable_asserts=False,
        num_devices=n_cores,
    )

    x_d = nc.dram_tensor("x", [T, D], f32, kind="ExternalInput").ap()
    wq_d = nc.dram_tensor("wq", [D, H], f32, kind="ExternalInput").ap()
    wk_d = nc.dram_tensor("wk", [D, H], f32, kind="ExternalInput").ap()
    wv_d = nc.dram_tensor("wv", [D, H], f32, kind="ExternalInput").ap()
    bq_d = nc.dram_tensor("bq", [H], f32, kind="ExternalInput").ap()
    bk_d = nc.dram_tensor("bk", [H], f32, kind="ExternalInput").ap()
    bv_d = nc.dram_tensor("bv", [H], f32, kind="ExternalInput").ap()
    out_d = nc.dram_tensor("out", [T, H], f32, kind="ExternalOutput").ap()

    x_ch = x_d.rearrange("(c p) d -> p c d", p=P)   # [128, 16, 768]
    out_tiles4 = out_d.rearrange("(n p) h -> p n h", p=P)

    scale = float(H) ** -0.5

    with tile.TileContext(nc) as tc, ExitStack() as ctx:
        const = ctx.enter_context(tc.tile_pool(name="const", bufs=1))
        big = ctx.enter_context(tc.tile_pool(name="big", bufs=1))
        xin = ctx.enter_context(tc.tile_pool(name="xin", bufs=1))
        work = ctx.enter_context(tc.tile_pool(name="work", bufs=1))
        pp = ctx.enter_context(tc.tile_pool(name="pp", bufs=1, space="PSUM"))

        # -- persistent activations (declared first so load_x can start
        #    desc-gen on the Pool engine before any other Pool work) -----
        xT = big.tile([P, DT, T], bf, tag="xT")          # xT[p, d, t] = x[t, 128d+p]
        qT = big.tile([H, T], bf, tag="qT")              # q^T [h, t]
        kT = big.tile([H, T], bf, tag="kT")              # k^T [h, t]
        v_sb = big.tile([P, TT, H + 1], bf, tag="v_sb")  # v natural + ones col

        x_half = {}

        def load_x_half(ch, half):
            # Half-chunk (2-tile) cast-DMAs: finer arrival granularity keeps
            # the kT-dependent score pairs fed without gen-pacing stalls.
            x_in = xin.tile([P, 2, D], bf, tag="x_in", bufs=8,
                            name=f"x_in_{ch}_{half}")
            nc.gpsimd.dma_start(x_in, x_ch[:, ds(4 * ch + 2 * half, 2), :])
            x_half[(ch, half)] = x_in

        load_x_half(0, 0)

        # -- constants / one-time setup ---------------------------------
        # PE p-state warmup: the Tensor engine ramps to full clock only
        # after ~3us of continuous work; a chain of dummy matmuls (fed by a
        # DVE-memset junk tile, ready at ~0.2us) bridges the gap until the
        # first x tiles arrive so the real transposes/projections run at
        # full speed.  Lives in the "sc" psum tag, unused until then.
        junk = work.tile([P, H], bf, tag="junk", name="junk")
        nc.vector.memset(junk, 0.0)
        wps = pp.tile([H, H], f32, tag="sc", bufs=2, name="warm")
        for i in range(48):
            nc.tensor.matmul(wps, junk, junk, start=True, stop=True,
                             skip_group_check=True)

        ident_f = const.tile([P, P], f32, tag="ident_f")
        make_identity(nc, ident_f)
        ident = const.tile([P, P], bf, tag="ident")
        nc.vector.tensor_copy(out=ident, in_=ident_f)

        # exp activation-table preload: tiny dummy exp at t~0 (the implicit
        # LoadActFuncSet preceding it has no operand deps and fires first)
        dum = work.tile([1, 4], f32, tag="dum", name="dum")
        nc.scalar.activation(dum, ident_f[0:1, 0:4],
                             mybir.ActivationFunctionType.Exp, scale=scale)

        # weights as bf16 via Pool cast-DMAs; emitted right after the chunk-0
        # x desc-gen in Pool program order (x chunk 0 keeps DMA priority, the
        # weights transfer next, chunks 1-3 follow).
        # wqk [128, 6, 0:64]=Wq, [.., 64:128]=Wk; wv [128, 6, 64]
        wqk = const.tile([P, DT, P], bf, tag="wqk")
        wv = const.tile([P, DT, H], bf, tag="wv")

        # Pool desc-gen order (= x/w DMA priority): first chunk-0 half, Wq,
        # second half, Wk, Wv, chunks 1-3.
        nc.gpsimd.dma_start(wqk[:, :, 0:H], wq_d.rearrange("(n p) h -> p n h", p=P))
        load_x_half(0, 1)
        nc.gpsimd.dma_start(wqk[:, :, H:P], wk_d.rearrange("(n p) h -> p n h", p=P))
        nc.gpsimd.dma_start(wv, wv_d.rearrange("(n p) h -> p n h", p=P))
        for c in (1, 2, 3):
            load_x_half(c, 0)
            load_x_half(c, 1)

        if biases:
            bias_qk = const.tile([P, 1], f32, tag="bias_qk")
            nc.sync.dma_start(bias_qk[0:H, :], bq_d[:, None])
            nc.sync.dma_start(bias_qk[H:P, :], bk_d[:, None])
            # bv broadcast to [128, 64] via K=1 matmul with a ones column
            bv_sb = const.tile([1, H], f32, tag="bv_sb")
            nc.sync.dma_start(bv_sb, bv_d[None, :])
            ones_col = const.tile([1, P], f32, tag="ones_col")
            nc.gpsimd.memset(ones_col, 1.0)
            ps_bv = pp.tile([P, H], f32, tag="proj", bufs=2, name="ps_bv")
            nc.tensor.matmul(ps_bv, ones_col, bv_sb, start=True, stop=True)
            bv_b = const.tile([P, H], f32, tag="bv_b")
            nc.vector.tensor_copy(out=bv_b, in_=ps_bv)

        nc.gpsimd.memset(v_sb[:, :, H : H + 1], 1.0)

        # -- per-chunk x transpose + projections ------------------------
        def transpose_tile(tt):
            ch, i = tt // 4, tt % 4
            src = x_half[(ch, i // 2)][:, i % 2, :]
            tr = pp.tile([P, DT, P], bf, tag="proj", bufs=2, name=f"tr_{tt}")
            for d in range(DT):
                nc.tensor.transpose(tr[:, d, :], src[:, ds(d * P, P)], ident)
            nc.vector.tensor_copy(out=xT[:, :, ts(tt, P)], in_=tr)

        def proj_qk(ch, half=None):
            # packed Q/K: psum rows 0:64 = q^T, 64:128 = k^T
            if half is None:
                w, sl = NCH, ds(ch * NCH, NCH)
                nm = f"qk_{ch}"
            else:
                w, sl = NCH // 2, ds(ch * NCH + half * (NCH // 2), NCH // 2)
                nm = f"qk_{ch}_{half}"
            ps = pp.tile([P, w], f32, tag="proj", bufs=2, name=nm)
            for d in range(DT):
                nc.tensor.matmul(ps, wqk[:, d, :], xT[:, d, sl],
                                 start=(d == 0), stop=(d == DT - 1))
            if biases:
                nc.vector.tensor_scalar_add(qT[:, sl], ps[0:H, :], bias_qk[0:H, :])
                nc.vector.tensor_scalar_add(kT[:, sl], ps[H:P, :], bias_qk[H:P, :])
            else:
                nc.vector.tensor_copy(out=kT[:, sl], in_=ps[H:P, :])
                nc.vector.tensor_copy(out=qT[:, sl], in_=ps[0:H, :])

        def proj_v(s0, ns):
            # V in natural [s, h] layout: lhsT = xT s-tile, rhs = Wv, N=64
            pv = pp.tile([P, ns, H], f32, tag="proj", bufs=2, name=f"v_{s0}")
            for j in range(ns):
                for d in range(DT):
                    nc.tensor.matmul(pv[:, j, :], xT[:, d, ts(s0 + j, P)],
                                     wv[:, d, :],
                                     start=(d == 0), stop=(d == DT - 1))
            nc.vector.tensor_copy(out=v_sb[:, ds(s0, ns), 0:H], in_=pv)

        # -- flash machinery --------------------------------------------
        ex_tiles = {}

        def scores_exp(fc, pr):
            s0, s1 = 2 * pr, 2 * pr + 1
            tsl = ds(fc * NCH, NCH)
            ps_s = pp.tile([P, 2, NCH], f32, tag="sc", bufs=2, name=f"sc_{fc}_{pr}")
            nc.tensor.matmul(ps_s[:, 0, :], kT[:, ts(s0, P)], qT[:, tsl],
                             start=True, stop=True)
            nc.tensor.matmul(ps_s[:, 1, :], kT[:, ts(s1, P)], qT[:, tsl],
                             start=True, stop=True)
            ex = work.tile([P, 2, NCH], bf, tag="ex", bufs=14, name=f"ex_{fc}_{pr}")
            nc.scalar.activation(ex, ps_s, mybir.ActivationFunctionType.Exp,
                                 scale=scale)
            ex_tiles[(fc, pr)] = ex

        def av_pair(fc, pr):
            # One PSUM bank holds all four jj slices.  start=True marks the
            # whole 2KB zero-region pending, so only the very FIRST matmul of
            # the fc may set it (each slice then auto-initializes on its first
            # write); a per-slice start would wipe sibling slices' partials.
            ex = ex_tiles.pop((fc, pr))
            for jj in range(4):
                for j in range(2):
                    s = 2 * pr + j
                    nc.tensor.matmul(
                        avo[fc][:, jj, :],
                        ex[:, j, ds(jj * P, P)],
                        v_sb[:, s, :],
                        start=(pr == 0 and j == 0 and jj == 0),
                        stop=(pr == NPR - 1 and j == 1),
                        skip_group_check=True,
                    )

        def epilogue(fc):
            # Last chunk is exit-critical: reciprocals on DVE and multiplies
            # on the (by then idle) Activation engine run pipelined, and the
            # output DMA goes in two halves so the first dispatches earlier.
            last = fc == NCC - 1
            ob = work.tile([P, 4, H], f32, tag="ob", bufs=2, name=f"ob_{fc}")
            for jj in range(4):
                rc = work.tile([P, 1], f32, tag="rc", bufs=4, name=f"rc_{fc}_{jj}")
                nc.vector.reciprocal(rc, avo[fc][:, jj, H : H + 1])
                if last:
                    nc.scalar.mul(ob[:, jj, :], avo[fc][:, jj, 0:H], rc)
                else:
                    nc.vector.tensor_scalar_mul(ob[:, jj, :], avo[fc][:, jj, 0:H], rc)
                if biases:
                    nc.vector.tensor_tensor(
                        out=ob[:, jj, :], in0=ob[:, jj, :], in1=bv_b,
                        op=mybir.AluOpType.add)
                if last and jj == 1:
                    nc.sync.dma_start(out_tiles4[:, ds(fc * 4, 2), :], ob[:, 0:2, :])
            if last:
                nc.sync.dma_start(out_tiles4[:, ds(fc * 4 + 2, 2), :], ob[:, 2:4, :])
            else:
                nc.sync.dma_start(out_tiles4[:, ts(fc, 4), :], ob)

        avo = {}

        def new_avo(fc):
            avo[fc] = pp.tile([P, 4, H + 1], f32, tag="avo", bufs=2, name=f"avo{fc}")

        # -- schedule ----------------------------------------------------
        # waves: pair (fc, pr) becomes computable after proj chunk
        # c = max(fc, pr // 2).  AV for fc >= 2 is deferred until an avo
        # PSUM bank frees (after epilogue(fc - 2)); ex tiles wait in SBUF.
        pend = []  # scores emitted, AV not yet emitted (lag hides Act latency)

        def flush_pend(n_keep=0):
            while len(pend) > n_keep:
                av_pair(*pend.pop(0))

        def emit_pair(fc, pr, defer_av=False):
            scores_exp(fc, pr)
            if defer_av:
                return
            pend.append((fc, pr))
            if len(pend) > 2:
                av_pair(*pend.pop(0))

        for ch in range(NCC):
            if ch == 0:
                # chunk 0: transposes first, one whole-width projection (the
                # first score pair needs the full qT chunk anyway)
                for tt in range(4):
                    transpose_tile(tt)
                proj_qk(0)
                proj_v(0, 2)
                proj_v(2, 2)
            else:
                for half in range(2):
                    transpose_tile(4 * ch + 2 * half)
                    transpose_tile(4 * ch + 2 * half + 1)
                    proj_qk(ch, half)
                    proj_v(4 * ch + 2 * half, 2)

            if ch < NCC - 1:
                # wave ch: all pairs with max(fc, pr//2) == ch, fc ascending.
                for fc in range(ch + 1):
                    if fc not in avo and fc < 2:
                        new_avo(fc)
                    prs = (range(2 * ch, 2 * ch + 2) if fc < ch
                           else range(0, 2 * ch + 2))
                    for pr in prs:
                        emit_pair(fc, pr, defer_av=(fc >= 2))

        # wave 3 (hand-ordered for Act continuity + early bank recycling)
        emit_pair(0, 6)
        emit_pair(0, 7)
        flush_pend()
        epilogue(0)

        emit_pair(1, 6)
        emit_pair(1, 7)
        flush_pend()
        epilogue(1)

        new_avo(2)  # reuses avo[0]'s bank
        for pr in range(6):
            pend.append((2, pr))  # ex already computed in wave 2
        flush_pend(n_keep=2)
        emit_pair(2, 6)
        emit_pair(2, 7)
        flush_pend()
        epilogue(2)

        new_avo(3)  # reuses avo[1]'s bank
        for pr in range(NPR):
            emit_pair(3, pr)
        flush_pend()
        epilogue(3)

    nc.compile()
    return nc


def _get_nc(mm="bf16", biases=False):
    key = (mm, biases)
    if key not in _CACHE:
        _CACHE[key] = _build(biases=biases)
    return _CACHE[key]


def kernel(x, Wq, bq, Wk, bk, Wv, bv, mm="bf16", **_kw):
    from concourse.bass_utils import run_bass_kernel_spmd

    x = np.ascontiguousarray(np.asarray(x, dtype=np.float32))
    base = {
        "wq": np.ascontiguousarray(np.asarray(Wq, np.float32)),
        "wk": np.ascontiguousarray(np.asarray(Wk, np.float32)),
        "wv": np.ascontiguousarray(np.asarray(Wv, np.float32)),
        "bq": np.ascontiguousarray(np.asarray(bq, np.float32)),
        "bk": np.ascontiguousarray(np.asarray(bk, np.float32)),
        "bv": np.ascontiguousarray(np.asarray(bv, np.float32)),
    }
    use_biases = bool(
        np.any(base["bq"]) or np.any(base["bk"]) or np.any(base["bv"])
    )
    nc = _get_nc(mm, biases=use_biases)
    in_maps = [dict(base, x=x[b]) for b in range(B)]
    res = run_bass_kernel_spmd(nc, in_maps, core_ids=list(range(B)))
    return np.stack([r["out"] for r in res.results], axis=0)


# revision 28
# speedup vs baseline: 1.1619x; 1.0157x over previous
"""Trainium2 Bass kernel for a single non-causal attention head.

Problem: x [8, 2048, 768] f32; Wq/Wk/Wv [768, 64]; bq/bk/bv [64].
  q = x@Wq+bq; k = x@Wk+bk; v = x@Wv+bv
  out = softmax(q k^T / sqrt(64)) @ v          -> [8, 2048, 64] f32

Sharding: data-parallel over batch B=8, one batch element per NeuronCore.

Per-core dataflow (all matmul operands bf16, fp32 accumulation in PSUM):
  1. x is loaded straight to bf16 via gpsimd (SWDGE) cast-DMAs, one DMA per
     512-row chunk (first chunk split in two for a faster pipeline start).
  2. Each 128-row x tile is PE-transposed (bf16, 1 cycle/row) into a PSUM
     tile and copied to the persistent xT [128d, 6, 2048t] (DVE 2x mode).
  3. Packed Q/K projection per 512-chunk: lhsT=[Wq|Wk] gives qT rows 0:64 /
     kT rows 64:128 in one 6-step accumulation; V is projected directly in
     natural [s, h] layout (lhsT = xT s-tile, rhs = Wv, N=64 -> 27ns/matmul)
     with a ones column appended so attention row-sums fall out of AV free.
  4. Flash loop over (fc t-chunk, pr s-pair): one [128, 2, 512] PSUM score
     tile (two K=64 matmuls), a single 1024-element exp on ScalarE (logit
     scale 1/8 folded in) -> ex bf16, then AV *transposed*: for each 128-t
     tile, matmul(out[t,65] += ex[s, t-slice].T @ v_sb[s-tile]) -- N=65, so
     the whole AV costs half of a streamed formulation AND the output lands
     in natural [t, h] layout: no epilogue transposes at all.
  5. Epilogue per (fc, jj): reciprocal of the sums column, per-partition
     scalar multiply -> ob, one DMA per 512-row block.

avo PSUM accumulators exist for 2 flash chunks at a time (8 PSUM banks
total); fc=2 AV work is deferred until epilogue(0) frees a bank, with the
already-computed ex tiles held in SBUF meanwhile, so the Activation engine
(the bottleneck: T*T exps = 27us floor) never stalls on PSUM.

Softmax is computed without the running-max subtraction: logits are q.k/8
with |logit| < ~3 for this problem's N(0,1)-scaled inputs, so exp is far
from overflow and the result matches jax.nn.softmax to bf16 accuracy.

Biases are all-zero in this problem; the default program skips them but
kernel() falls back to a bias-applying variant if any bias is nonzero.
"""

import numpy as np

B, T, D, H = 8, 2048, 768, 64
P = 128
DT = D // P   # 6 d-tiles
TT = T // P   # 16 s/t-tiles
NCH = 512     # t-chunk width
NCC = T // NCH  # 4 chunks
NPR = TT // 2   # 8 s-pairs

_CACHE = {}


def _build(biases=False, n_cores=8):
    from contextlib import ExitStack

    import concourse.bass as bass
    import concourse.tile as tile
    from concourse import bacc, mybir
    from concourse.bass import ds, ts
    from concourse.masks import make_identity

    f32 = mybir.dt.float32
    bf = mybir.dt.bfloat16

    nc = bacc.Bacc(
        "TRN2",
        target_bir_lowering=False,
        debug=False,
        enable_asserts=False,
        num_devices=n_cores,
    )

    x_d = nc.dram_tensor("x", [T, D], f32, kind="ExternalInput").ap()
    wq_d = nc.dram_tensor("wq", [D, H], f32, kind="ExternalInput").ap()
    wk_d = nc.dram_tensor("wk", [D, H], f32, kind="ExternalInput").ap()
    wv_d = nc.dram_tensor("wv", [D, H], f32, kind="ExternalInput").ap()
    bq_d = nc.dram_tensor("bq", [H], f32, kind="ExternalInput").ap()
    bk_d = nc.dram_tensor("bk", [H], f32, kind="ExternalInput").ap()
    bv_d = nc.dram_tensor("bv", [H], f32, kind="ExternalInput").ap()
    out_d = nc.dram_tensor("out", [T, H], f32, kind="ExternalOutput").ap()

    x_ch = x_d.rearrange("(c p) d -> p c d", p=P)   # [128, 16, 768]
    out_tiles4 = out_d.rearrange("(n p) h -> p n h", p=P)

    scale = float(H) ** -0.5

    with tile.TileContext(nc) as tc, ExitStack() as ctx:
        const = ctx.enter_context(tc.tile_pool(name="const", bufs=1))
        big = ctx.enter_context(tc.tile_pool(name="big", bufs=1))
        xin = ctx.enter_context(tc.tile_pool(name="xin", bufs=1))
        work = ctx.enter_context(tc.tile_pool(name="work", bufs=1))
        pp = ctx.enter_context(tc.tile_pool(name="pp", bufs=1, space="PSUM"))

        # -- persistent activations (declared first so load_x can start
        #    desc-gen on the Pool engine before any other Pool work) -----
        xT = big.tile([P, DT, T], bf, tag="xT")          # xT[p, d, t] = x[t, 128d+p]
        qT = big.tile([H, T], bf, tag="qT")              # q^T [h, t]
        kT = big.tile([H, T], bf, tag="kT")              # k^T [h, t]
        v_sb = big.tile([P, TT, H + 1], bf, tag="v_sb")  # v natural + ones col

        x_half = {}

        def load_x_half(ch, half):
            # Half-chunk (2-tile) cast-DMAs: finer arrival granularity keeps
            # the kT-dependent score pairs fed without gen-pacing stalls.
            x_in = xin.tile([P, 2, D], bf, tag="x_in", bufs=8,
                            name=f"x_in_{ch}_{half}")
            nc.gpsimd.dma_start(x_in, x_ch[:, ds(4 * ch + 2 * half, 2), :])
            x_half[(ch, half)] = x_in

        load_x_half(0, 0)

        # -- constants / one-time setup ---------------------------------
        # PE p-state warmup: the Tensor engine ramps to full clock only
        # after ~3us of continuous work; a chain of dummy matmuls (fed by a
        # DVE-memset junk tile, ready at ~0.2us) bridges the gap until the
        # first x tiles arrive so the real transposes/projections run at
        # full speed.  Lives in the "sc" psum tag, unused until then.
        junk = work.tile([P, H], bf, tag="junk", name="junk")
        nc.vector.memset(junk, 0.0)
        wps = pp.tile([H, H], f32, tag="sc", bufs=2, name="warm")
        for i in range(48):
            nc.tensor.matmul(wps, junk, junk, start=True, stop=True,
                             skip_group_check=True)

        ident_f = const.tile([P, P], f32, tag="ident_f")
        make_identity(nc, ident_f)
        ident = const.tile([P, P], bf, tag="ident")
        nc.vector.tensor_copy(out=ident, in_=ident_f)

        # exp activation-table preload: tiny dummy exp at t~0 (the implicit
        # LoadActFuncSet preceding it has no operand deps and fires first)
        dum = work.tile([1, 4], f32, tag="dum", name="dum")
        nc.scalar.activation(dum, ident_f[0:1, 0:4],
                             mybir.ActivationFunctionType.Exp, scale=scale)

        # weights as bf16 via Pool cast-DMAs; emitted right after the chunk-0
        # x desc-gen in Pool program order (x chunk 0 keeps DMA priority, the
        # weights transfer next, chunks 1-3 follow).
        # wqk [128, 6, 0:64]=Wq, [.., 64:128]=Wk; wv [128, 6, 64]
        wqk = const.tile([P, DT, P], bf, tag="wqk")
        wv = const.tile([P, DT, H], bf, tag="wv")

        # Pool desc-gen order (= x/w DMA priority): first chunk-0 half, Wq,
        # second half, Wk, Wv, chunks 1-3.
        nc.gpsimd.dma_start(wqk[:, :, 0:H], wq_d.rearrange("(n p) h -> p n h", p=P))
        load_x_half(0, 1)
        nc.gpsimd.dma_start(wqk[:, :, H:P], wk_d.rearrange("(n p) h -> p n h", p=P))
        nc.gpsimd.dma_start(wv, wv_d.rearrange("(n p) h -> p n h", p=P))
        for c in (1, 2, 3):
            load_x_half(c, 0)
            load_x_half(c, 1)

        if biases:
            bias_qk = const.tile([P, 1], f32, tag="bias_qk")
            nc.sync.dma_start(bias_qk[0:H, :], bq_d[:, None])
            nc.sync.dma_start(bias_qk[H:P, :], bk_d[:, None])
            # bv broadcast to [128, 64] via K=1 matmul with a ones column
            bv_sb = const.tile([1, H], f32, tag="bv_sb")
            nc.sync.dma_start(bv_sb, bv_d[None, :])
            ones_col = const.tile([1, P], f32, tag="ones_col")
            nc.gpsimd.memset(ones_col, 1.0)
            ps_bv = pp.tile([P, H], f32, tag="proj", bufs=2, name="ps_bv")
            nc.tensor.matmul(ps_bv, ones_col, bv_sb, start=True, stop=True)
            bv_b = const.tile([P, H], f32, tag="bv_b")
            nc.vector.tensor_copy(out=bv_b, in_=ps_bv)

        nc.gpsimd.memset(v_sb[:, :, H : H + 1], 1.0)

        # -- per-chunk x transpose + projections ------------------------
        def transpose_tile(tt):
            ch, i = tt // 4, tt % 4
            src = x_half[(ch, i // 2)][:, i % 2, :]
            tr = pp.tile([P, DT, P], bf, tag="proj", bufs=2, name=f"tr_{tt}")
            for d in range(DT):
                nc.tensor.transpose(tr[:, d, :], src[:, ds(d * P, P)], ident)
            nc.vector.tensor_copy(out=xT[:, :, ts(tt, P)], in_=tr)

        def proj_qk(ch, half=None):
            # packed Q/K: psum rows 0:64 = q^T, 64:128 = k^T
            if half is None:
                w, sl = NCH, ds(ch * NCH, NCH)
                nm = f"qk_{ch}"
            else:
                w, sl = NCH // 2, ds(ch * NCH + half * (NCH // 2), NCH // 2)
                nm = f"qk_{ch}_{half}"
            ps = pp.tile([P, w], f32, tag="proj", bufs=2, name=nm)
            for d in range(DT):
                nc.tensor.matmul(ps, wqk[:, d, :], xT[:, d, sl],
                                 start=(d == 0), stop=(d == DT - 1))
            if biases:
                nc.vector.tensor_scalar_add(qT[:, sl], ps[0:H, :], bias_qk[0:H, :])
                nc.vector.tensor_scalar_add(kT[:, sl], ps[H:P, :], bias_qk[H:P, :])
            else:
                nc.vector.tensor_copy(out=kT[:, sl], in_=ps[H:P, :])
                nc.vector.tensor_copy(out=qT[:, sl], in_=ps[0:H, :])

        def proj_v(s0, ns):
            # V in natural [s, h] layout: lhsT = xT s-tile, rhs = Wv, N=64
            pv = pp.tile([P, ns, H], f32, tag="proj", bufs=2, name=f"v_{s0}")
            for j in range(ns):
                for d in range(DT):
                    nc.tensor.matmul(pv[:, j, :], xT[:, d, ts(s0 + j, P)],
                                     wv[:, d, :],
                                     start=(d == 0), stop=(d == DT - 1))
            nc.vector.tensor_copy(out=v_sb[:, ds(s0, ns), 0:H], in_=pv)

        # -- flash machinery --------------------------------------------
        ex_tiles = {}

        def scores_exp(fc, pr):
            s0, s1 = 2 * pr, 2 * pr + 1
            tsl = ds(fc * NCH, NCH)
            ps_s = pp.tile([P, 2, NCH], f32, tag="sc", bufs=2, name=f"sc_{fc}_{pr}")
            nc.tensor.matmul(ps_s[:, 0, :], kT[:, ts(s0, P)], qT[:, tsl],
                             start=True, stop=True)
            nc.tensor.matmul(ps_s[:, 1, :], kT[:, ts(s1, P)], qT[:, tsl],
                             start=True, stop=True)
            ex = work.tile([P, 2, NCH], bf, tag="ex", bufs=14, name=f"ex_{fc}_{pr}")
            nc.scalar.activation(ex, ps_s, mybir.ActivationFunctionType.Exp,
                                 scale=scale)
            ex_tiles[(fc, pr)] = ex

        def av_pair(fc, pr):
            # One PSUM bank holds all four jj slices.  start=True marks the
            # whole 2KB zero-region pending, so only the very FIRST matmul of
            # the fc may set it (each slice then auto-initializes on its first
            # write); a per-slice start would wipe sibling slices' partials.
            ex = ex_tiles.pop((fc, pr))
            for jj in range(4):
                for j in range(2):
                    s = 2 * pr + j
                    nc.tensor.matmul(
                        avo[fc][:, jj, :],
                        ex[:, j, ds(jj * P, P)],
                        v_sb[:, s, :],
                        start=(pr == 0 and j == 0 and jj == 0),
                        stop=(pr == NPR - 1 and j == 1),
                        skip_group_check=True,
                    )

        def epilogue(fc):
            # Last chunk is exit-critical: reciprocals on DVE and multiplies
            # on the (by then idle) Activation engine run pipelined, and the
            # output DMA goes in two halves so the first dispatches earlier.
            last = fc == NCC - 1
            ob = work.tile([P, 4, H], f32, tag="ob", bufs=2, name=f"ob_{fc}")
            for jj in range(4):
                rc = work.tile([P, 1], f32, tag="rc", bufs=4, name=f"rc_{fc}_{jj}")
                nc.vector.reciprocal(rc, avo[fc][:, jj, H : H + 1])
                if last:
                    nc.scalar.mul(ob[:, jj, :], avo[fc][:, jj, 0:H], rc)
                else:
                    nc.vector.tensor_scalar_mul(ob[:, jj, :], avo[fc][:, jj, 0:H], rc)
                if biases:
                    nc.vector.tensor_tensor(
                        out=ob[:, jj, :], in0=ob[:, jj, :], in1=bv_b,
                        op=mybir.AluOpType.add)
                if last and jj == 1:
                    nc.sync.dma_start(out_tiles4[:, ds(fc * 4, 2), :], ob[:, 0:2, :])
            if last:
                nc.sync.dma_start(out_tiles4[:, ds(fc * 4 + 2, 2), :], ob[:, 2:4, :])
            else:
                nc.sync.dma_start(out_tiles4[:, ts(fc, 4), :], ob)

        avo = {}

        def new_avo(fc):
            avo[fc] = pp.tile([P, 4, H + 1], f32, tag="avo", bufs=2, name=f"avo{fc}")

        # -- schedule ----------------------------------------------------
        # waves: pair (fc, pr) becomes computable after proj chunk
        # c = max(fc, pr // 2).  AV for fc >= 2 is deferred until an avo
        # PSUM bank frees (after epilogue(fc - 2)); ex tiles wait in SBUF.
        pend = []  # scores emitted, AV not yet emitted (lag hides Act latency)

        def flush_pend(n_keep=0):
            while len(pend) > n_keep:
                av_pair(*pend.pop(0))

        def emit_pair(fc, pr, defer_av=False):
            scores_exp(fc, pr)
            if defer_av:
                return
            pend.append((fc, pr))
            if len(pend) > 2:
                av_pair(*pend.pop(0))

        for ch in range(NCC):
            # The x->xT->qT/kT production ladder gates each wave's exps; at
            # high priority it preempts the (plentiful) score/AV queue work.
            with tc.high_priority():
                if ch == 0:
                    # chunk 0: transposes first, one whole-width projection
                    # (the first score pair needs the full qT chunk anyway)
                    for tt in range(4):
                        transpose_tile(tt)
                    proj_qk(0)
                    proj_v(0, 2)
                    proj_v(2, 2)
                else:
                    for half in range(2):
                        transpose_tile(4 * ch + 2 * half)
                        transpose_tile(4 * ch + 2 * half + 1)
                        proj_qk(ch, half)
                        proj_v(4 * ch + 2 * half, 2)

            if ch < NCC - 1:
                # wave ch: all pairs with max(fc, pr//2) == ch, fc ascending.
                for fc in range(ch + 1):
                    if fc not in avo and fc < 2:
                        new_avo(fc)
                    prs = (range(2 * ch, 2 * ch + 2) if fc < ch
                           else range(0, 2 * ch + 2))
                    for pr in prs:
                        emit_pair(fc, pr, defer_av=(fc >= 2))

        # wave 3 (hand-ordered for Act continuity + early bank recycling)
        emit_pair(0, 6)
        emit_pair(0, 7)
        flush_pend()
        epilogue(0)

        emit_pair(1, 6)
        emit_pair(1, 7)
        flush_pend()
        epilogue(1)

        new_avo(2)  # reuses avo[0]'s bank
        for pr in range(6):
            pend.append((2, pr))  # ex already computed in wave 2
        flush_pend(n_keep=2)
        emit_pair(2, 6)
        emit_pair(2, 7)
        flush_pend()
        epilogue(2)

        new_avo(3)  # reuses avo[1]'s bank
        for pr in range(NPR):
            emit_pair(3, pr)
        flush_pend()
        epilogue(3)

    nc.compile()
    return nc


def _get_nc(mm="bf16", biases=False):
    key = (mm, biases)
    if key not in _CACHE:
        _CACHE[key] = _build(biases=biases)
    return _CACHE[key]


def kernel(x, Wq, bq, Wk, bk, Wv, bv, mm="bf16", **_kw):
    from concourse.bass_utils import run_bass_kernel_spmd

    x = np.ascontiguousarray(np.asarray(x, dtype=np.float32))
    base = {
        "wq": np.ascontiguousarray(np.asarray(Wq, np.float32)),
        "wk": np.ascontiguousarray(np.asarray(Wk, np.float32)),
        "wv": np.ascontiguousarray(np.asarray(Wv, np.float32)),
        "bq": np.ascontiguousarray(np.asarray(bq, np.float32)),
        "bk": np.ascontiguousarray(np.asarray(bk, np.float32)),
        "bv": np.ascontiguousarray(np.asarray(bv, np.float32)),
    }
    use_biases = bool(
        np.any(base["bq"]) or np.any(base["bk"]) or np.any(base["bv"])
    )
    nc = _get_nc(mm, biases=use_biases)
    in_maps = [dict(base, x=x[b]) for b in range(B)]
    res = run_bass_kernel_spmd(nc, in_maps, core_ids=list(range(B)))
    return np.stack([r["out"] for r in res.results], axis=0)
